# revision 5
# baseline (speedup 1.0000x reference)
"""Trainium2 Bass kernel for nn_AbsoluteNeuralLayer.

Reference computation:
    classical = x @ classical_weights + classical_biases          # [B, DOUT]
    probs[j]  = |scan of circulant "rotations" applied to s0|[0]^2
    out       = tanh(classical + probs[None, :])

Key simplification: the scan state s0 is a constant vector, and every step
maps a constant vector to a constant vector scaled by cos(angle)
(s_new[i] = cos*s - sin*s + sin*s = cos*s elementwise).  Hence
    probs[j] = (prod_{t<48} cos(ang[j, t]))^2 / DIN
with ang[j, 3*d+g] = absolute_weights[d, j, g] for g < 3.

Sharding (8 cores): batch split 4 ways x dout split 2 ways.  Each core
computes out[1024 batch rows, 1024 dout cols] as tanh(x_s @ W_s + bias_s +
probs_s) with dout on PSUM partitions and batch on the moving free dim
(fp32r matmuls: fp32 accuracy at 16-bit PE streaming rate), accumulating
over K=2048 in 16 k-tiles.  probs+bias are computed once per core on
ACT/DVE (tiny) and applied as the per-partition bias of the Tanh
activation that drains PSUM.  Outputs are written transposed and
un-transposed on the host during the gather.

Schedule: the PE only has 8 PSUM banks, so the 16 output tiles are
processed in two n-half phases of 8 concurrent accumulation groups:
  phase 1: n-tiles 0-3  x both batch chunks   (needs W cols 0-511 + all x)
  phase 2: n-tiles 4-7  x both batch chunks   (W cols 512-1023 stream
           behind phase 1; x already resident)
All inputs are host-packed into SBUF layout so every DMA is a single
contiguous-per-partition transfer; chunk sizes taper up front (fast
start) and phase-2 W arrives in 2-k-tile chunks (fine-grained pacing).
"""

import math

import numpy as np

import concourse.bacc as bacc
import concourse.mybir as mybir
from concourse.tile import TileContext
from concourse.bass_utils import run_bass_kernel_spmd

B, DIN, DOUT, DEPTH = 4096, 2048, 2048, 16
NCORES = 8
BB, DB = 4, 2            # batch blocks x dout blocks (BB*DB == NCORES)
MB, NB = B // BB, DOUT // DB   # per-core batch rows (1024) / dout cols (1024)
KT = DIN // 128          # 16 contraction tiles
NT = NB // 128           # 8 dout tiles
NH = NB // 2             # 512 cols per n-half
MCH = 512                # batch chunk = one PSUM bank of fp32
MC = MB // MCH           # 2 chunks
NANG = 3 * DEPTH         # 48 angles per output column

PH1_CHUNKS = [2, 2, 4, 4, 4]     # k-tiles per DMA chunk, phase-1 stream
PH2_CHUNKS = [2] * 8             # phase-2 W stream

F32 = mybir.dt.float32
F32R = mybir.dt.float32r
AF = mybir.ActivationFunctionType

_NC_CACHE = None


def _chunk_offsets(chunks):
    off, out = 0, []
    for c in chunks:
        out.append((off, c))
        off += c
    return out


def _build():
    nc = bacc.Bacc("TRN2", target_bir_lowering=False, debug=False, num_devices=NCORES)
    # host-packed SBUF layouts:
    #   wb  [p, (g*KT + k)*NH + n'] = W[128k+p, g*NH + n']   (g = n-half)
    #   xb  [p, (u*KT + k)*MCH + m] = x[u*MCH + m, 128k+p]   (u = m-chunk)
    wb = nc.dram_tensor("wb", [128, 2 * KT * NH], F32R, kind="ExternalInput")
    xb = nc.dram_tensor("xb", [128, MC * KT * MCH], F32R, kind="ExternalInput")
    ang = nc.dram_tensor("ang", [128, NT * NANG], F32, kind="ExternalInput")
    bias = nc.dram_tensor("bias", [128, NT], F32, kind="ExternalInput")
    outT = nc.dram_tensor("outT", [NB, MB], F32, kind="ExternalOutput")

    with TileContext(nc) as tc:
        with (
            tc.tile_pool(name="big", bufs=1) as big,
            tc.tile_pool(name="small", bufs=1) as small,
            tc.tile_pool(name="outp", bufs=4) as outp,
            tc.tile_pool(name="psum", bufs=1, space="PSUM") as psump,
        ):
            # ---- phase-1 stream: W n-half 0 + x (both chunks), k-chunked ----
            wg = [[None] * KT for _ in range(2)]   # wg[g][k] -> (tile, col offset)
            xs = [[None] * KT for _ in range(MC)]  # xs[u][k]
            for ci, (k0, kn) in enumerate(_chunk_offsets(PH1_CHUNKS)):
                wt = big.tile([128, kn * NH], F32R, tag=f"w0_{ci}", name=f"w0_{ci}")
                nc.sync.dma_start(out=wt, in_=wb[:, k0 * NH:(k0 + kn) * NH])
                for i in range(kn):
                    wg[0][k0 + i] = (wt, i * NH)
                for u in range(MC):
                    xt = big.tile([128, kn * MCH], F32R, tag=f"x{u}_{ci}", name=f"x{u}_{ci}")
                    nc.sync.dma_start(
                        out=xt,
                        in_=xb[:, (u * KT + k0) * MCH:(u * KT + k0 + kn) * MCH],
                    )
                    for i in range(kn):
                        xs[u][k0 + i] = (xt, i * MCH)

            # ---- phase-2 W stream (behind phase 1) ----
            for ci, (k0, kn) in enumerate(_chunk_offsets(PH2_CHUNKS)):
                wt = big.tile([128, kn * NH], F32R, tag=f"w1_{ci}", name=f"w1_{ci}")
                nc.sync.dma_start(
                    out=wt, in_=wb[:, (KT + k0) * NH:(KT + k0 + kn) * NH]
                )
                for i in range(kn):
                    wg[1][k0 + i] = (wt, i * NH)

            # ---- probs + bias (tiny, ACT/DVE) ----
            ang_sb = small.tile([128, NT * NANG], F32, tag="ang")
            nc.sync.dma_start(out=ang_sb, in_=ang[:, :])
            bias_sb = small.tile([128, NT], F32, tag="bias")
            nc.sync.dma_start(out=bias_sb, in_=bias[:, :])
            halfpi = small.tile([128, 1], F32, tag="halfpi")
            nc.any.memset(halfpi, math.pi / 2)
            cos_sb = small.tile([128, NT * NANG], F32, tag="cos")
            nc.scalar.activation(cos_sb, ang_sb, AF.Sin, bias=halfpi)

            def v3(t):
                return t.rearrange("p (a b) -> p a b", a=NT)

            t24 = small.tile([128, NT * 24], F32, tag="t24")
            nc.vector.tensor_mul(v3(t24), v3(cos_sb)[:, :, 0:24], v3(cos_sb)[:, :, 24:48])
            t12 = small.tile([128, NT * 12], F32, tag="t12")
            nc.vector.tensor_mul(v3(t12), v3(t24)[:, :, 0:12], v3(t24)[:, :, 12:24])
            t6 = small.tile([128, NT * 6], F32, tag="t6")
            nc.vector.tensor_mul(v3(t6), v3(t12)[:, :, 0:6], v3(t12)[:, :, 6:12])
            t3 = small.tile([128, NT * 3], F32, tag="t3")
            nc.vector.tensor_mul(v3(t3), v3(t6)[:, :, 0:3], v3(t6)[:, :, 3:6])
            t1 = small.tile([128, NT], F32, tag="t1")
            nc.vector.tensor_mul(v3(t1), v3(t3)[:, :, 0:1], v3(t3)[:, :, 1:2])
            nc.vector.tensor_mul(v3(t1), v3(t1), v3(t3)[:, :, 2:3])
            sq = small.tile([128, NT], F32, tag="sq")
            nc.vector.tensor_mul(sq, t1, t1)
            nc.vector.tensor_scalar_mul(sq, sq, 1.0 / DIN)
            btot = small.tile([128, NT], F32, tag="btot")
            nc.vector.tensor_add(btot, sq, bias_sb)

            def mm_w(g, k, t):
                # lhsT [128, 128] for n-tile (g*4 + t), k-tile k
                wt, off = wg[g][k]
                return wt[:, off + 128 * t:off + 128 * (t + 1)]

            def mm_x(u, k):
                xt, off = xs[u][k]
                return xt[:, off:off + MCH]

            def epilogue(n, ps_tile, u):
                o = outp.tile([128, MCH], F32, tag="o", name=f"o{n}_{u}")
                nc.scalar.activation(o, ps_tile, AF.Tanh, bias=btot[:, n:n + 1])
                # out DMA issued from the scalar engine's HWDGE ring so it
                # doesn't queue behind the input stream on the sync ring
                nc.scalar.dma_start(
                    out=outT[128 * n:128 * (n + 1), u * MCH:(u + 1) * MCH], in_=o
                )

            # ---- phase 1: groups (n-tile t in 0-3) x (m-chunk u) ----
            ps1 = {
                (t, u): psump.tile(
                    [128, MCH], F32, tag=f"ps{t * 2 + u}", name=f"ps1_{t}_{u}"
                )
                for t in range(4)
                for u in range(MC)
            }
            for k in range(KT):
                for t in range(4):
                    for u in range(MC):
                        nc.tensor.matmul(
                            ps1[(t, u)], mm_w(0, k, t), mm_x(u, k),
                            start=(k == 0), stop=(k == KT - 1),
                        )
            for t in range(4):
                for u in range(MC):
                    epilogue(t, ps1[(t, u)], u)

            # ---- phase 2: n-tiles 4-7, two sub-phases for tail stagger ----
            for sub in range(2):
                ps2 = {
                    (t, u): psump.tile(
                        [128, MCH], F32,
                        tag=f"ps{(sub * 2 + t) * 2 + u}",
                        name=f"ps2_{sub}_{t}_{u}",
                    )
                    for t in range(2)
                    for u in range(MC)
                }
                for k in range(KT):
                    for t in range(2):
                        for u in range(MC):
                            nc.tensor.matmul(
                                ps2[(t, u)], mm_w(1, k, sub * 2 + t), mm_x(u, k),
                                start=(k == 0), stop=(k == KT - 1),
                            )
                for t in range(2):
                    for u in range(MC):
                        epilogue(4 + sub * 2 + t, ps2[(t, u)], u)

    nc.compile()
    return nc


def _get_nc():
    global _NC_CACHE
    if _NC_CACHE is None:
        _NC_CACHE = _build()
    return _NC_CACHE


def _in_map_for_core(core, x, absolute_weights, classical_weights, classical_biases):
    i, j = core % BB, core // BB
    rows = slice(i * MB, (i + 1) * MB)
    cols = slice(j * NB, (j + 1) * NB)
    # wb[p, (g*KT + k)*NH + n'] = W[128k+p, g*NH+n']
    ws = classical_weights[:, cols].reshape(KT, 128, 2, NH)   # [k, p, g, n']
    wbm = np.ascontiguousarray(ws.transpose(1, 2, 0, 3).reshape(128, 2 * KT * NH))
    # xb[p, (u*KT + k)*MCH + m] = x[rows][u*MCH+m, 128k+p]
    xsT = x[rows, :].T                                        # [DIN, MB] view
    xr = xsT.reshape(KT, 128, MC, MCH)                        # [k, p, u, m]
    xbm = np.ascontiguousarray(xr.transpose(1, 2, 0, 3).reshape(128, MC * KT * MCH))
    # ang[j_local, 3*d+g] = absolute_weights[d, j, g]
    angj = np.transpose(absolute_weights[:, cols, :3], (1, 0, 2)).reshape(NB, NANG)
    ang_sb = np.ascontiguousarray(
        angj.reshape(NT, 128, NANG).transpose(1, 0, 2).reshape(128, NT * NANG)
    )
    bias_sb = np.ascontiguousarray(classical_biases[cols].reshape(NT, 128).T)
    return {
        "wb": wbm.astype(np.float32, copy=False),
        "xb": xbm.astype(np.float32, copy=False),
        "ang": ang_sb.astype(np.float32, copy=False),
        "bias": bias_sb.astype(np.float32, copy=False),
    }


def kernel(x, absolute_weights, classical_weights, classical_biases, **_ignored):
    x = np.asarray(x, dtype=np.float32)
    absolute_weights = np.asarray(absolute_weights, dtype=np.float32)
    classical_weights = np.asarray(classical_weights, dtype=np.float32)
    classical_biases = np.asarray(classical_biases, dtype=np.float32)

    nc = _get_nc()
    in_maps = [
        _in_map_for_core(c, x, absolute_weights, classical_weights, classical_biases)
        for c in range(NCORES)
    ]
    res = run_bass_kernel_spmd(nc, in_maps, list(range(NCORES)))

    out = np.empty((B, DOUT), np.float32)
    for c in range(NCORES):
        i, j = c % BB, c // BB
        out[i * MB:(i + 1) * MB, j * NB:(j + 1) * NB] = res.results[c]["outT"].T
    return out


# revision 6
# speedup vs baseline: 1.2198x; 1.2198x over previous
"""Trainium2 Bass kernel for nn_AbsoluteNeuralLayer.

Reference computation:
    classical = x @ classical_weights + classical_biases          # [B, DOUT]
    probs[j]  = |scan of circulant "rotations" applied to s0|[0]^2
    out       = tanh(classical + probs[None, :])

Key simplification: the scan state s0 is a constant vector, and every step
maps a constant vector to a constant vector scaled by cos(angle)
(s_new[i] = cos*s - sin*s + sin*s = cos*s elementwise).  Hence
    probs[j] = (prod_{t<48} cos(ang[j, t]))^2 / DIN
with ang[j, 3*d+g] = absolute_weights[d, j, g] for g < 3.

Sharding (8 cores): batch split 4 ways x dout split 2 ways.  Each core
computes out[1024 batch rows, 1024 dout cols] as tanh(x_s @ W_s + bias_s +
probs_s) with dout on PSUM partitions and batch on the moving free dim
(fp32r matmuls: fp32 accuracy at 16-bit PE streaming rate), accumulating
over K=2048 in 16 k-tiles.  probs+bias are computed once per core on
ACT/DVE (tiny) and applied as the per-partition bias of the Tanh
activation that drains PSUM.  Outputs are written transposed and
un-transposed on the host during the gather.

Schedule (8 PSUM banks → 8 concurrent accumulation groups):
  pass A: all 8 n-tiles x batch-chunk 0, k-outer; DMA-paced while W + x0
          stream in (tapered chunks so the first matmul starts ~3 us in).
  pass B: batch-chunk 1 in two n-halves; B's first matmuls only wait for
          pass A's first epilogues (same banks), so the PE never idles
          long enough for the HAM to re-throttle, and the second half's
          epilogues are the only tail.
All inputs are host-packed into SBUF layout so every DMA is a single
contiguous-per-partition transfer at ~370 GB/s.
"""

import math

import numpy as np

import concourse.bacc as bacc
import concourse.mybir as mybir
from concourse.tile import TileContext
from concourse.bass_utils import run_bass_kernel_spmd

B, DIN, DOUT, DEPTH = 4096, 2048, 2048, 16
NCORES = 8
BB, DB = 4, 2            # batch blocks x dout blocks (BB*DB == NCORES)
MB, NB = B // BB, DOUT // DB   # per-core batch rows (1024) / dout cols (1024)
KT = DIN // 128          # 16 contraction tiles
NT = NB // 128           # 8 dout tiles
MCH = 512                # batch chunk = one PSUM bank of fp32
MC = MB // MCH           # 2 chunks
NANG = 3 * DEPTH         # 48 angles per output column

A_CHUNKS = [1, 1, 2, 4, 4, 4]   # k-tiles per DMA chunk for the pass-A stream
B_CHUNKS = [2, 2, 4, 4, 4]      # x1 stream

F32 = mybir.dt.float32
F32R = mybir.dt.float32r
AF = mybir.ActivationFunctionType

_NC_CACHE = None


def _chunk_offsets(chunks):
    off, out = 0, []
    for c in chunks:
        out.append((off, c))
        off += c
    return out


def _build():
    nc = bacc.Bacc("TRN2", target_bir_lowering=False, debug=False, num_devices=NCORES)
    # host-packed SBUF layouts:
    #   wb [p, k*NB + n]          = W[128k+p, n]
    #   xb [p, (u*KT + k)*MCH+m]  = x[u*MCH + m, 128k+p]   (u = m-chunk)
    wb = nc.dram_tensor("wb", [128, KT * NB], F32R, kind="ExternalInput")
    xb = nc.dram_tensor("xb", [128, MC * KT * MCH], F32R, kind="ExternalInput")
    ang = nc.dram_tensor("ang", [128, NT * NANG], F32, kind="ExternalInput")
    bias = nc.dram_tensor("bias", [128, NT], F32, kind="ExternalInput")
    outT = nc.dram_tensor("outT", [NB, MB], F32, kind="ExternalOutput")

    with TileContext(nc) as tc:
        with (
            tc.tile_pool(name="big", bufs=1) as big,
            tc.tile_pool(name="small", bufs=1) as small,
            tc.tile_pool(name="outp", bufs=4) as outp,
            tc.tile_pool(name="psum", bufs=1, space="PSUM") as psump,
        ):
            # ---- pass-A stream: W (full) + x chunk 0, k-chunked, tapered ----
            wg = [None] * KT   # (tile, col offset) per k
            xs = [[None] * KT for _ in range(MC)]
            for ci, (k0, kn) in enumerate(_chunk_offsets(A_CHUNKS)):
                wt = big.tile([128, kn * NB], F32R, tag=f"w{ci}", name=f"w{ci}")
                nc.sync.dma_start(out=wt, in_=wb[:, k0 * NB:(k0 + kn) * NB])
                for i in range(kn):
                    wg[k0 + i] = (wt, i * NB)
                xt = big.tile([128, kn * MCH], F32R, tag=f"x0_{ci}", name=f"x0_{ci}")
                nc.sync.dma_start(out=xt, in_=xb[:, k0 * MCH:(k0 + kn) * MCH])
                for i in range(kn):
                    xs[0][k0 + i] = (xt, i * MCH)

            # ---- probs + bias (tiny, ACT/DVE) ----
            ang_sb = small.tile([128, NT * NANG], F32, tag="ang")
            nc.sync.dma_start(out=ang_sb, in_=ang[:, :])
            bias_sb = small.tile([128, NT], F32, tag="bias")
            nc.sync.dma_start(out=bias_sb, in_=bias[:, :])
            halfpi = small.tile([128, 1], F32, tag="halfpi")
            nc.any.memset(halfpi, math.pi / 2)
            cos_sb = small.tile([128, NT * NANG], F32, tag="cos")
            nc.scalar.activation(cos_sb, ang_sb, AF.Sin, bias=halfpi)

            def v3(t):
                return t.rearrange("p (a b) -> p a b", a=NT)

            t24 = small.tile([128, NT * 24], F32, tag="t24")
            nc.vector.tensor_mul(v3(t24), v3(cos_sb)[:, :, 0:24], v3(cos_sb)[:, :, 24:48])
            t12 = small.tile([128, NT * 12], F32, tag="t12")
            nc.vector.tensor_mul(v3(t12), v3(t24)[:, :, 0:12], v3(t24)[:, :, 12:24])
            t6 = small.tile([128, NT * 6], F32, tag="t6")
            nc.vector.tensor_mul(v3(t6), v3(t12)[:, :, 0:6], v3(t12)[:, :, 6:12])
            t3 = small.tile([128, NT * 3], F32, tag="t3")
            nc.vector.tensor_mul(v3(t3), v3(t6)[:, :, 0:3], v3(t6)[:, :, 3:6])
            t1 = small.tile([128, NT], F32, tag="t1")
            nc.vector.tensor_mul(v3(t1), v3(t3)[:, :, 0:1], v3(t3)[:, :, 1:2])
            nc.vector.tensor_mul(v3(t1), v3(t1), v3(t3)[:, :, 2:3])
            sq = small.tile([128, NT], F32, tag="sq")
            nc.vector.tensor_mul(sq, t1, t1)
            nc.vector.tensor_scalar_mul(sq, sq, 1.0 / DIN)
            btot = small.tile([128, NT], F32, tag="btot")
            nc.vector.tensor_add(btot, sq, bias_sb)

            def mm_w(k, n):
                wt, off = wg[k]
                return wt[:, off + 128 * n:off + 128 * (n + 1)]

            def mm_x(u, k):
                xt, off = xs[u][k]
                return xt[:, off:off + MCH]

            def epilogue(n, ps_tile, u):
                o = outp.tile([128, MCH], F32, tag="o", name=f"o{n}_{u}")
                nc.scalar.activation(o, ps_tile, AF.Tanh, bias=btot[:, n:n + 1])
                # out DMA from the scalar engine's HWDGE ring: doesn't queue
                # behind the input stream on the sync ring
                nc.scalar.dma_start(
                    out=outT[128 * n:128 * (n + 1), u * MCH:(u + 1) * MCH], in_=o
                )

            # ---- pass A: m-chunk 0, k-outer over 8 PSUM groups ----
            psA = [
                psump.tile([128, MCH], F32, tag=f"ps{n}", name=f"psA{n}")
                for n in range(NT)
            ]
            for k in range(KT):
                for n in range(NT):
                    nc.tensor.matmul(
                        psA[n], mm_w(k, n), mm_x(0, k),
                        start=(k == 0), stop=(k == KT - 1),
                    )

            # x chunk 1 stream (lands behind pass A)
            for ci, (k0, kn) in enumerate(_chunk_offsets(B_CHUNKS)):
                xt = big.tile([128, kn * MCH], F32R, tag=f"x1_{ci}", name=f"x1_{ci}")
                nc.sync.dma_start(
                    out=xt, in_=xb[:, (KT + k0) * MCH:(KT + k0 + kn) * MCH]
                )
                for i in range(kn):
                    xs[1][k0 + i] = (xt, i * MCH)

            # pass A epilogues (ACT) — free banks in n order for pass B
            for n in range(NT):
                epilogue(n, psA[n], 0)

            # ---- pass B: m-chunk 1, two n-halves for staggered tail ----
            for h in range(2):
                psB = [
                    psump.tile(
                        [128, MCH], F32, tag=f"ps{h * 4 + t}", name=f"psB{h * 4 + t}"
                    )
                    for t in range(4)
                ]
                for k in range(KT):
                    for t in range(4):
                        nc.tensor.matmul(
                            psB[t], mm_w(k, h * 4 + t), mm_x(1, k),
                            start=(k == 0), stop=(k == KT - 1),
                        )
                for t in range(4):
                    epilogue(h * 4 + t, psB[t], 1)

    nc.compile()
    return nc


def _get_nc():
    global _NC_CACHE
    if _NC_CACHE is None:
        _NC_CACHE = _build()
    return _NC_CACHE


def _in_map_for_core(core, x, absolute_weights, classical_weights, classical_biases):
    i, j = core % BB, core // BB
    rows = slice(i * MB, (i + 1) * MB)
    cols = slice(j * NB, (j + 1) * NB)
    # wb[p, k*NB + n] = W[128k+p, n]
    wbm = np.ascontiguousarray(
        classical_weights[:, cols].reshape(KT, 128, NB).transpose(1, 0, 2).reshape(128, KT * NB)
    )
    # xb[p, (u*KT + k)*MCH + m] = x[rows][u*MCH+m, 128k+p]
    xsT = x[rows, :].T                                        # [DIN, MB] view
    xr = xsT.reshape(KT, 128, MC, MCH)                        # [k, p, u, m]
    xbm = np.ascontiguousarray(xr.transpose(1, 2, 0, 3).reshape(128, MC * KT * MCH))
    # ang[j_local, 3*d+g] = absolute_weights[d, j, g]
    angj = np.transpose(absolute_weights[:, cols, :3], (1, 0, 2)).reshape(NB, NANG)
    ang_sb = np.ascontiguousarray(
        angj.reshape(NT, 128, NANG).transpose(1, 0, 2).reshape(128, NT * NANG)
    )
    bias_sb = np.ascontiguousarray(classical_biases[cols].reshape(NT, 128).T)
    return {
        "wb": wbm.astype(np.float32, copy=False),
        "xb": xbm.astype(np.float32, copy=False),
        "ang": ang_sb.astype(np.float32, copy=False),
        "bias": bias_sb.astype(np.float32, copy=False),
    }


def kernel(x, absolute_weights, classical_weights, classical_biases, **_ignored):
    x = np.asarray(x, dtype=np.float32)
    absolute_weights = np.asarray(absolute_weights, dtype=np.float32)
    classical_weights = np.asarray(classical_weights, dtype=np.float32)
    classical_biases = np.asarray(classical_biases, dtype=np.float32)

    nc = _get_nc()
    in_maps = [
        _in_map_for_core(c, x, absolute_weights, classical_weights, classical_biases)
        for c in range(NCORES)
    ]
    res = run_bass_kernel_spmd(nc, in_maps, list(range(NCORES)))

    out = np.empty((B, DOUT), np.float32)
    for c in range(NCORES):
        i, j = c % BB, c // BB
        out[i * MB:(i + 1) * MB, j * NB:(j + 1) * NB] = res.results[c]["outT"].T
    return out
